# revision 1
# baseline (speedup 1.0000x reference)
"""Multi-head attention, tensor-parallel across 8 Trainium2 NeuronCores.

Sharding: core = (batch b, head-group g) with b in {0,1}, g in {0..3}.
Each core computes 4 heads (a 256-wide slice of the head dimension) for one
batch element:
  Q^T/K^T = Wq/Wk slice^T-projections of query/key (kept transposed: [dh, s])
  V       = value @ Wv slice (natural [s, dh]), with an appended ones column
  S^T     = K^T-chunk.T @ Q^T-chunk per head  -> scores transposed [j, i]
  E       = exp(S^T * scale)                  (no max subtraction; scores ~N(0,1))
  [O^T;Z] = V'.T @ E  accumulated over j      (ones column yields Z = sum_j E)
  Onorm^T = O^T * (1/Z) broadcast
  outT    = Wo-slice.T @ Onorm^T (+ bo on group-0 cores only)
Host: transposes activations into [D, S] per core, and sums the 4 group
partials per batch (the "all-reduce" of the output projection), then
transposes back.

Inputs arrive full-size; all sharding is internal.
"""

import numpy as np

# Problem shape (hardcoded per the harness contract).
B, S, D, H = 2, 2048, 1024, 16
DK = D // H              # 64 head dim
N_CORES = 8
GROUPS = N_CORES // B    # 4 head-groups
DH = D // GROUPS         # 256 head-dims per core (4 heads)
H_CORE = DH // DK        # 4 heads per core
SCALE = 1.0 / float(np.sqrt(DK))

P = 128                  # SBUF/PSUM partitions
SC = 512                 # matmul moving-dim chunk (one PSUM bank of fp32)
IB = 1024                # flash i-block (exp granule)


def build_nc(S=S, D=D, DH=DH, DK=DK, scale=SCALE, ib=IB, dtype="f32r"):
    """Build the per-core Bass module (same NEFF for all 8 cores)."""
    import concourse.bacc as bacc
    import concourse.mybir as mybir
    import concourse.tile as tile

    f32 = mybir.dt.float32
    f32r = mybir.dt.float32r
    bf16 = mybir.dt.bfloat16
    Exp = mybir.ActivationFunctionType.Exp

    KT = D // P                    # contraction tiles for projections
    NSC = S // SC                  # s chunks
    HC = DH // P                   # head-dim chunks (2)
    HPC = P // DK                  # heads per chunk (2)
    H_CORE = DH // DK
    JT = S // P                    # j tiles
    NIB = S // ib                  # i blocks
    ICB = ib // SC                 # i chunks per block
    NOUT = D // P                  # output row chunks

    cdt = {"f32r": f32r, "bf16": bf16, "f32": f32}[dtype]

    def mm(ap):
        return ap

    nc = bacc.Bacc("TRN2", target_bir_lowering=False, debug=False)

    qT = nc.dram_tensor("qT", [D, S], cdt, kind="ExternalInput")
    kTd = nc.dram_tensor("kTd", [D, S], cdt, kind="ExternalInput")
    vT = nc.dram_tensor("vT", [D, S], cdt, kind="ExternalInput")
    wq = nc.dram_tensor("wq", [D, DH], cdt, kind="ExternalInput")
    wk = nc.dram_tensor("wk", [D, DH], cdt, kind="ExternalInput")
    wv = nc.dram_tensor("wv", [D, DH], cdt, kind="ExternalInput")
    wo = nc.dram_tensor("wo", [DH, D], cdt, kind="ExternalInput")
    bq = nc.dram_tensor("bq", [P, HC], f32, kind="ExternalInput")
    bk = nc.dram_tensor("bk", [P, HC], f32, kind="ExternalInput")
    bvb = nc.dram_tensor("bvb", [P, H_CORE, DK], f32, kind="ExternalInput")
    bo = nc.dram_tensor("bo", [P, NOUT], f32, kind="ExternalInput")
    outT = nc.dram_tensor("outT", [D, S], f32, kind="ExternalOutput")

    with tile.TileContext(nc) as tc:
        with (
            tc.tile_pool(name="const", bufs=1) as cpool,
            tc.tile_pool(name="pers", bufs=1) as pers,
            tc.tile_pool(name="stream", bufs=1) as stream,
            tc.tile_pool(name="psum", bufs=1, space="PSUM") as psum,
            tc.tile_pool(name="dscratch", bufs=1, space="DRAM") as dscratch,
        ):
            # ---- constants ----
            wq_sb = cpool.tile([P, KT, DH], cdt, name="wq_sb")
            wk_sb = cpool.tile([P, KT, DH], cdt, name="wk_sb")
            wv_sb = cpool.tile([P, KT, DH], cdt, name="wv_sb")
            wo_sb = cpool.tile([P, HC, D], cdt, name="wo_sb")
            bq_sb = cpool.tile([P, HC], f32, name="bq_sb")
            bk_sb = cpool.tile([P, HC], f32, name="bk_sb")
            bvb_sb = cpool.tile([P, H_CORE, DK], f32, name="bvb_sb")
            bo_sb = cpool.tile([P, NOUT], f32, name="bo_sb")
            nc.sync.dma_start(wq_sb[:], qT_ap_rearr(wq, P))
            nc.sync.dma_start(wk_sb[:], qT_ap_rearr(wk, P))
            nc.sync.dma_start(wv_sb[:], qT_ap_rearr(wv, P))
            nc.sync.dma_start(wo_sb[:], wo[:, :].rearrange("(c p) n -> p c n", p=P))
            nc.sync.dma_start(bq_sb[:], bq[:, :])
            nc.sync.dma_start(bk_sb[:], bk[:, :])
            nc.sync.dma_start(bvb_sb[:], bvb[:, :, :])
            nc.sync.dma_start(bo_sb[:], bo[:, :])

            # ---- persistent activations ----
            # Q^T/K^T live per head on partitions 64-127 (base-64 K=64
            # matmuls sustain full rate; base-0 ones run at half rate).
            qt_h = [pers.tile([P, S], cdt, name=f"qth{h}")
                    for h in range(H_CORE)]
            kt_h = [pers.tile([P, S], cdt, name=f"kth{h}")
                    for h in range(H_CORE)]
            v_c = [pers.tile([P, JT, HPC, DK + 1], cdt, name=f"v{c}") for c in range(HC)]
            on_c = [pers.tile([P, S], cdt, name=f"on{c}") for c in range(HC)]

            for c in range(HC):
                ones_ap = v_c[c][:, :, :, DK:DK + 1]
                if dtype == "f32r":
                    ones_ap = ones_ap.bitcast(f32)
                nc.vector.memset(ones_ap, 1.0)

            # ---- projections ----
            def qk_proj(src, w_sb, b_sb, dst, chunks):
                for si in range(NSC):
                    ins = []
                    for kt in range(KT):
                        t = stream.tile([P, SC], cdt, tag="instream", bufs=12,
                                        name=f"in_{src.name}_{si}_{kt}_{chunks[0]}")
                        nc.sync.dma_start(
                            t[:], src[kt * P:(kt + 1) * P,
                                      si * SC:(si + 1) * SC])
                        ins.append(t)
                        yield
                    for c in chunks:
                        ps = psum.tile([P, SC], f32, tag="mm", bufs=4,
                                       name=f"ps_{src.name}_{si}_{c}")
                        for kt in range(KT):
                            nc.tensor.matmul(
                                ps[:],
                                lhsT=mm(w_sb[:, kt, c * P:(c + 1) * P]),
                                rhs=mm(ins[kt][:]),
                                start=(kt == 0), stop=(kt == KT - 1))
                            yield
                        stg = stream.tile([P, SC], cdt, tag="pstage", bufs=3,
                                          name=f"stg_{src.name}_{si}_{c}")
                        nc.vector.tensor_add(
                            stg[:], ps[:],
                            b_sb[:, c:c + 1].to_broadcast((P, SC)))
                        ssl = slice(si * SC, (si + 1) * SC)
                        nc.sync.dma_start(dst[c * HPC][DK:P, ssl],
                                          stg[0:DK, :])
                        nc.sync.dma_start(dst[c * HPC + 1][DK:P, ssl],
                                          stg[DK:P, :])
                        yield

            for g in (qk_proj(qT, wq_sb, bq_sb, qt_h, tuple(range(HC))),
                      qk_proj(kTd, wk_sb, bk_sb, kt_h, tuple(range(HC)))):
                for _ in g:
                    pass
            deferred = iter(())

            # V natural: psum[s, dh] = sum_k vT[k, s] * Wv[k, dh]
            for si in range(NSC):
                ins = []
                for kt in range(KT):
                    t = stream.tile([P, SC], cdt, tag="instream", bufs=12,
                                    name=f"in_v_{si}_{kt}")
                    nc.sync.dma_start(
                        t[:], vT[kt * P:(kt + 1) * P, si * SC:(si + 1) * SC])
                    ins.append(t)
                for sub in range(SC // P):
                    jt_idx = si * (SC // P) + sub
                    ps = psum.tile([P, DH], f32, tag="mm", bufs=4,
                                   name=f"ps_v_{jt_idx}")
                    for kt in range(KT):
                        nc.tensor.matmul(
                            ps[:],
                            lhsT=mm(ins[kt][:, sub * P:(sub + 1) * P]),
                            rhs=mm(wv_sb[:, kt, :]),
                            start=(kt == 0), stop=(kt == KT - 1))
                    for c in range(HC):
                        nc.vector.tensor_add(
                            v_c[c][:, jt_idx, :, 0:DK],
                            ps[:, c * P:(c + 1) * P].rearrange(
                                "p (h d) -> p h d", d=DK),
                            bvb_sb[:, c * HPC:(c + 1) * HPC, :])

            # ---- attention (flash over j, scores transposed) ----
            # Per-head blocks; sc has two buffers so scores(jt+1) overlap
            # exp(jt). AV matmuls trail one j-step so the PE program never
            # blocks the ACT engine behind unready work.
            for h in range(H_CORE):
                hc = h // HPC
                hh = h % HPC
                p0 = hh * DK
                for ibx in range(NIB):
                    i0 = ibx * ib
                    avs = [
                        psum.tile([P, SC], f32, tag="mm", bufs=4,
                                  name=f"av_{h}_{ibx}_{ic}")
                        for ic in range(ICB)
                    ]
                    e_ts = {}
                    for jt in range(JT + 1):
                        if jt < JT:
                            sc_t = psum.tile([P, ib], f32, tag="sc",
                                             bufs=2,
                                             name=f"sc_{h}_{ibx}_{jt}")
                            for ic in range(ICB):
                                nc.tensor.matmul(
                                    sc_t[:, ic * SC:(ic + 1) * SC],
                                    lhsT=mm(kt_h[h][DK:P,
                                                    jt * P:(jt + 1) * P]),
                                    rhs=mm(qt_h[h][DK:P,
                                                   i0 + ic * SC:i0 + (ic + 1) * SC]),
                                    start=True, stop=True)
                            e_t = stream.tile([P, ib], cdt, tag="e", bufs=3,
                                              name=f"e_{h}_{ibx}_{jt}")
                            nc.scalar.activation(e_t[:], sc_t[:], Exp,
                                                 bias=0.0, scale=scale)
                            e_ts[jt] = e_t
                        if jt >= 1:
                            pj = jt - 1
                            e_t = e_ts.pop(pj)
                            for ic in range(ICB):
                                nc.tensor.matmul(
                                    avs[ic][0:DK + 1, :],
                                    lhsT=mm(v_c[hc][:, pj, hh, :]),
                                    rhs=mm(e_t[:, ic * SC:(ic + 1) * SC]),
                                    start=(pj == 0), stop=(pj == JT - 1))
                    # drain AV psums to SBUF, normalize in the background
                    for ic in range(ICB):
                        av = avs[ic]
                        av_sb = stream.tile([P, SC], f32, tag="avsb", bufs=4,
                                            name=f"avsb_{h}_{ibx}_{ic}")
                        nc.vector.tensor_copy(av_sb[0:DK + 1, :],
                                              av[0:DK + 1, :])
                        rz = stream.tile([P, SC], f32, tag="rz", bufs=2,
                                         name=f"rz_{h}_{ibx}_{ic}")
                        nc.vector.reciprocal(rz[DK:DK + 1, :],
                                             av_sb[DK:DK + 1, :])
                        rz_d = dscratch.tile([1, SC], f32, tag="rzd", bufs=2,
                                             name=f"rzd_{h}_{ibx}_{ic}")
                        nc.sync.dma_start(rz_d[:], rz[DK:DK + 1, :])
                        rzb = stream.tile([P, SC], f32, tag="rzb", bufs=2,
                                          name=f"rzb_{h}_{ibx}_{ic}")
                        nc.sync.dma_start(
                            rzb[0:DK, :],
                            rz_d[:, :].to_broadcast((DK, SC)))
                        ot = stream.tile([P, SC], cdt, tag="ot", bufs=2,
                                         name=f"ot_{h}_{ibx}_{ic}")
                        nc.vector.tensor_mul(ot[0:DK, :], av_sb[0:DK, :],
                                             rzb[0:DK, :])
                        nc.sync.dma_start(
                            on_c[hc][p0:p0 + DK,
                                     i0 + ic * SC:i0 + (ic + 1) * SC],
                            ot[0:DK, :])

            # ---- output projection ----
            Ident = mybir.ActivationFunctionType.Identity
            for n in range(NOUT):
                for i in range(NSC):
                    idx = n * NSC + i
                    ps = psum.tile([P, SC], f32, tag=("sc", "mm")[idx % 2],
                                   bufs=(2, 4)[idx % 2],
                                   name=f"ps_o_{n}_{i}")
                    for c in range(HC):
                        nc.tensor.matmul(
                            ps[:],
                            lhsT=mm(wo_sb[:, c, n * P:(n + 1) * P]),
                            rhs=mm(on_c[c][:, i * SC:(i + 1) * SC]),
                            start=(c == 0), stop=(c == HC - 1))
                    o_sb = stream.tile([P, SC], f32, tag="osb", bufs=4,
                                       name=f"o_sb_{n}_{i}")
                    if idx % 2 == 0:
                        nc.scalar.activation(o_sb[:], ps[:], Ident,
                                             bias=bo_sb[:, n:n + 1],
                                             scale=1.0)
                    else:
                        nc.vector.tensor_add(
                            o_sb[:], ps[:],
                            bo_sb[:, n:n + 1].to_broadcast((P, SC)))
                    nc.sync.dma_start(
                        outT[n * P:(n + 1) * P, i * SC:(i + 1) * SC], o_sb[:])

    nc.finalize()
    return nc


def qT_ap_rearr(w_dram, p):
    """[D, N] dram weight -> [P, D//P, N] AP for SBUF load."""
    return w_dram[:, :].rearrange("(ko p) n -> p ko n", p=p)


def make_in_maps(query, key, value, Wq, bq, Wk, bk, Wv, bv, Wo, bo,
                 dtype="f32r"):
    """Shard full inputs into the 8 per-core input dicts."""
    f = lambda a: np.ascontiguousarray(np.asarray(a, dtype=np.float32))
    HC = DH // P
    NOUT = D // P
    query, key, value = f(query), f(key), f(value)
    Wq, Wk, Wv, Wo = f(Wq), f(Wk), f(Wv), f(Wo)
    bq, bk, bv, bo = f(bq), f(bk), f(bv), f(bo)
    if dtype == "bf16":
        import ml_dtypes
        cvt = lambda a: np.ascontiguousarray(a.astype(ml_dtypes.bfloat16))
    else:
        cvt = np.ascontiguousarray
    in_maps = []
    for core in range(N_CORES):
        b, g = core // GROUPS, core % GROUPS
        sl = slice(g * DH, (g + 1) * DH)
        in_maps.append({
            "qT": cvt(query[b].T),
            "kTd": cvt(key[b].T),
            "vT": cvt(value[b].T),
            "wq": cvt(Wq[:, sl]),
            "wk": cvt(Wk[:, sl]),
            "wv": cvt(Wv[:, sl]),
            "wo": cvt(Wo[sl, :]),
            "bq": np.ascontiguousarray(bq[sl].reshape(HC, P).T),
            "bk": np.ascontiguousarray(bk[sl].reshape(HC, P).T),
            "bvb": np.ascontiguousarray(
                np.broadcast_to(bv[sl].reshape(H_CORE, DK)[None], (P, H_CORE, DK))),
            "bo": (np.ascontiguousarray(bo.reshape(NOUT, P).T)
                   if g == 0 else np.zeros((P, NOUT), np.float32)),
        })
    return in_maps


# test hooks (ignored by the harness)
TRACE = False
LAST_RESULT = None
DTYPE = "bf16"
_NC_CACHE = {}


def kernel(query, key, value, Wq, bq, Wk, bk, Wv, bv, Wo, bo):
    global LAST_RESULT
    from concourse.bass_utils import run_bass_kernel_spmd

    if DTYPE not in _NC_CACHE:
        _NC_CACHE[DTYPE] = build_nc(dtype=DTYPE)
    nc = _NC_CACHE[DTYPE]

    in_maps = make_in_maps(query, key, value, Wq, bq, Wk, bk, Wv, bv, Wo, bo,
                           dtype=DTYPE)
    kwargs = {}
    if TRACE:
        kwargs = dict(trace=True, trace_cores=[0])
    res = run_bass_kernel_spmd(nc, in_maps, core_ids=list(range(N_CORES)), **kwargs)
    LAST_RESULT = res

    out = np.zeros((B, S, D), np.float32)
    for core in range(N_CORES):
        b = core // GROUPS
        out[b] += res.results[core]["outT"].T
    return out



# revision 2
# speedup vs baseline: 1.2808x; 1.2808x over previous
"""Multi-head attention, tensor-parallel across 8 Trainium2 NeuronCores.

Sharding: core = (batch b, head-group g) with b in {0,1}, g in {0..3}.
Each core computes 4 heads (a 256-wide slice of the head dimension) for one
batch element:
  Q^T/K^T = Wq/Wk slice^T-projections of query/key (kept transposed: [dh, s])
  V       = value @ Wv slice (natural [s, dh]), with an appended ones column
  S^T     = K^T-chunk.T @ Q^T-chunk per head  -> scores transposed [j, i]
  E       = exp(S^T * scale)                  (no max subtraction; scores ~N(0,1))
  [O^T;Z] = V'.T @ E  accumulated over j      (ones column yields Z = sum_j E)
  Onorm^T = O^T * (1/Z) broadcast
  outT    = Wo-slice.T @ Onorm^T (+ bo on group-0 cores only)
Host: transposes activations into [D, S] per core, pre-arranges weights into
partition-major layout (so every DMA line is >=2KB), and sums the 4 group
partials per batch (the "all-reduce" of the output projection).

Schedule (per core):
  - All input DMAs issue up front in priority order (wk/wq, kT, qT, wv, vT, wo)
    so the Sync engine never head-of-line-blocks a load behind a dependent
    store; transfers use full/half row granules (2-4KB per-partition lines).
  - K/Q projections accumulate kt-major so the PE starts as rows land.
  - Attention is ACT(exp)-bound: head 0 runs a "lookahead" block that emits
    16 scores+exp first (deep e-tile buffering) so the exp stream starts
    before V has even arrived; V projection chunks and both i-blocks' AV
    matmuls are interleaved behind it.
  - Softmax denominators are normalized via a packed [128,8] reciprocal
    (1/Z gathered through DRAM) instead of 1-partition reciprocal ops.
"""

import numpy as np

# Problem shape (hardcoded per the harness contract).
B, S, D, H = 2, 2048, 1024, 16
DK = D // H              # 64 head dim
N_CORES = 8
GROUPS = N_CORES // B    # 4 head-groups
DH = D // GROUPS         # 256 head-dims per core (4 heads)
H_CORE = DH // DK        # 4 heads per core
SCALE = 1.0 / float(np.sqrt(DK))

P = 128                  # SBUF/PSUM partitions
SC = 512                 # matmul moving-dim chunk (one PSUM bank of fp32)
IB = 1024                # flash i-block / exp granule / DMA granule


def build_nc(dtype="bf16"):
    """Build the per-core Bass module (same NEFF for all 8 cores)."""
    import concourse.bacc as bacc
    import concourse.mybir as mybir
    import concourse.tile as tile

    f32 = mybir.dt.float32
    f32r = mybir.dt.float32r
    bf16 = mybir.dt.bfloat16
    Exp = mybir.ActivationFunctionType.Exp
    Ident = mybir.ActivationFunctionType.Identity

    KT = D // P                    # 8 contraction tiles for projections
    HC = DH // P                   # 2 head-dim chunks
    HPC = P // DK                  # 2 heads per chunk
    JT = S // P                    # 16 j tiles
    NIB = S // IB                  # 2 i blocks
    SJ = S // IB                   # 2 column halves (DMA/proj granule)
    NOUT = D // P                  # 8 output row chunks
    XB = IB // SC                  # 2 moving chunks per i block
    ZF = IB // P                   # 8 z values per partition when packed

    cdt = {"f32r": f32r, "bf16": bf16, "f32": f32}[dtype]
    odt = f32 if dtype == "f32" else bf16

    nc = bacc.Bacc("TRN2", target_bir_lowering=False, debug=False)

    qT = nc.dram_tensor("qT", [D, S], cdt, kind="ExternalInput")
    kTd = nc.dram_tensor("kTd", [D, S], cdt, kind="ExternalInput")
    vT = nc.dram_tensor("vT", [D, S], cdt, kind="ExternalInput")
    wq = nc.dram_tensor("wq", [P, KT, DH], cdt, kind="ExternalInput")
    wk = nc.dram_tensor("wk", [P, KT, DH], cdt, kind="ExternalInput")
    wv = nc.dram_tensor("wv", [P, KT, DH], cdt, kind="ExternalInput")
    wo = nc.dram_tensor("wo", [P, HC, D], cdt, kind="ExternalInput")
    bq = nc.dram_tensor("bq", [P, HC], f32, kind="ExternalInput")
    bk = nc.dram_tensor("bk", [P, HC], f32, kind="ExternalInput")
    bvb = nc.dram_tensor("bvb", [P, H_CORE, DK], f32, kind="ExternalInput")
    bo = nc.dram_tensor("bo", [P, NOUT], f32, kind="ExternalInput")
    outT = nc.dram_tensor("outT", [D, S], odt, kind="ExternalOutput")

    with tile.TileContext(nc) as tc:
        with (
            tc.tile_pool(name="const", bufs=1) as cpool,
            tc.tile_pool(name="pers", bufs=1) as pers,
            tc.tile_pool(name="stream", bufs=1) as stream,
            tc.tile_pool(name="psum", bufs=1, space="PSUM") as psum,
            tc.tile_pool(name="dscratch", bufs=1, space="DRAM") as dscratch,
        ):
            # ---- ACT exp-table warm-up (overlaps the input DMA stream) ----
            warm = stream.tile([1, 8], f32, name="warm")
            warm_o = stream.tile([1, 8], f32, name="warm_o")
            nc.vector.memset(warm[:], 0.0)
            nc.scalar.activation(warm_o[:], warm[:], Exp, bias=0.0, scale=1.0)

            # ---- constants + inputs, DMA'd in priority order ----
            wk_sb = cpool.tile([P, KT, DH], cdt, name="wk_sb")
            wq_sb = cpool.tile([P, KT, DH], cdt, name="wq_sb")
            wv_sb = cpool.tile([P, KT, DH], cdt, name="wv_sb")
            wo_sb = cpool.tile([P, HC, D], cdt, name="wo_sb")
            bq_sb = cpool.tile([P, HC], f32, name="bq_sb")
            bk_sb = cpool.tile([P, HC], f32, name="bk_sb")
            bvb_sb = cpool.tile([P, H_CORE, DK], f32, name="bvb_sb")
            bo_sb = cpool.tile([P, NOUT], f32, name="bo_sb")

            nc.sync.dma_start(wk_sb[:], wk[:, :, :])
            nc.sync.dma_start(wq_sb[:], wq[:, :, :])
            nc.sync.dma_start(bk_sb[:], bk[:, :])
            nc.sync.dma_start(bq_sb[:], bq[:, :])

            # kT halves (tag shared with vT rows: vT reuses the slots after
            # the K projection has consumed them)
            krow = [[None] * SJ for _ in range(KT)]
            for kt in range(KT):
                for hf in range(SJ):
                    t = stream.tile([P, IB], cdt, tag="xrow", bufs=16,
                                    name=f"krow{kt}_{hf}")
                    nc.sync.dma_start(
                        t[:], kTd[kt * P:(kt + 1) * P, hf * IB:(hf + 1) * IB])
                    krow[kt][hf] = t

            # qT halves: all of half 0 first so Q-proj wave A starts early
            qrow = [[None] * KT for _ in range(SJ)]
            for hf in range(SJ):
                for kt in range(KT):
                    t = stream.tile([P, IB], cdt, tag="qrow", bufs=16,
                                    name=f"qrow{kt}_{hf}")
                    nc.sync.dma_start(
                        t[:], qT[kt * P:(kt + 1) * P, hf * IB:(hf + 1) * IB])
                    qrow[hf][kt] = t

            nc.sync.dma_start(wv_sb[:], wv[:, :, :])
            nc.sync.dma_start(bvb_sb[:], bvb[:, :, :])
            vrow = [[None] * SJ for _ in range(KT)]
            for hf in range(SJ):
                for kt in range(KT):
                    t = stream.tile([P, IB], cdt, tag="xrow", bufs=16,
                                    name=f"vrow{kt}_{hf}")
                    nc.sync.dma_start(
                        t[:], vT[kt * P:(kt + 1) * P, hf * IB:(hf + 1) * IB])
                    vrow[kt][hf] = t

            nc.sync.dma_start(wo_sb[:], wo[:, :, :])
            nc.sync.dma_start(bo_sb[:], bo[:, :])

            # ---- persistent activations ----
            # Q^T/K^T live per head on partitions 64-127 (base-64 K=64
            # matmuls sustain full rate; base-0 ones run at half rate).
            qt_h = [pers.tile([P, S], cdt, name=f"qth{h}")
                    for h in range(H_CORE)]
            kt_h = [pers.tile([P, S], cdt, name=f"kth{h}")
                    for h in range(H_CORE)]
            v_c = [pers.tile([P, JT, HPC, DK + 1], cdt, name=f"v{c}")
                   for c in range(HC)]
            on_c = [pers.tile([P, S], cdt, name=f"on{c}") for c in range(HC)]

            for c in range(HC):
                ones_ap = v_c[c][:, :, :, DK:DK + 1]
                if dtype == "f32r":
                    ones_ap = ones_ap.bitcast(f32)
                nc.vector.memset(ones_ap, 1.0)

            # ---- K/Q projections (kt-major accumulation per column half) ----
            def qk_proj(rows, w_sb, b_sb, dst, pname):
                for sj in range(SJ):
                    pss = []
                    for c in range(HC):
                        ps = psum.tile([P, IB], f32, tag=("sc", "av")[c],
                                       bufs=2, name=f"ps_{pname}_{sj}_{c}")
                        pss.append(ps)
                    for kt in range(KT):
                        for c in range(HC):
                            for x in range(XB):
                                nc.tensor.matmul(
                                    pss[c][:, x * SC:(x + 1) * SC],
                                    lhsT=w_sb[:, kt, c * P:(c + 1) * P],
                                    rhs=rows(kt, sj)[:, x * SC:(x + 1) * SC],
                                    start=(kt == 0), stop=(kt == KT - 1))
                    for c in range(HC):
                        stg = stream.tile([P, IB], cdt, tag="stg", bufs=3,
                                          name=f"stg_{pname}_{sj}_{c}")
                        nc.vector.tensor_add(
                            stg[:], pss[c][:],
                            b_sb[:, c:c + 1].to_broadcast((P, IB)))
                        ssl = slice(sj * IB, (sj + 1) * IB)
                        nc.sync.dma_start(dst[c * HPC][DK:P, ssl],
                                          stg[0:DK, :])
                        nc.sync.dma_start(dst[c * HPC + 1][DK:P, ssl],
                                          stg[DK:P, :])

            qk_proj(lambda kt, sj: krow[kt][sj], wk_sb, bk_sb, kt_h, "k")
            qk_proj(lambda kt, sj: qrow[sj][kt], wq_sb, bq_sb, qt_h, "q")

            # ---- V projection (natural [s, dh]), emitted in chunks that are
            # interleaved into head 0's attention stream ----
            def v_proj_chunk(jts):
                for jt in jts:
                    hf, sub = divmod(jt, IB // P)
                    ps = psum.tile([P, DH], f32, tag="sc", bufs=2,
                                   name=f"ps_v_{jt}")
                    for kt in range(KT):
                        nc.tensor.matmul(
                            ps[:],
                            lhsT=vrow[kt][hf][:, sub * P:(sub + 1) * P],
                            rhs=wv_sb[:, kt, :],
                            start=(kt == 0), stop=(kt == KT - 1))
                    for c in range(HC):
                        nc.vector.tensor_add(
                            v_c[c][:, jt, :, 0:DK],
                            ps[:, c * P:(c + 1) * P].rearrange(
                                "p (h d) -> p h d", d=DK),
                            bvb_sb[:, c * HPC:(c + 1) * HPC, :])

            # ---- attention (flash over j, scores transposed) ----
            def scores_exp(h, ibx, jt, e_ts):
                i0 = ibx * IB
                sc_t = psum.tile([P, IB], f32, tag="sc", bufs=2,
                                 name=f"sc_{h}_{ibx}_{jt}")
                for x in range(XB):
                    nc.tensor.matmul(
                        sc_t[:, x * SC:(x + 1) * SC],
                        lhsT=kt_h[h][DK:P, jt * P:(jt + 1) * P],
                        rhs=qt_h[h][DK:P, i0 + x * SC:i0 + (x + 1) * SC],
                        start=True, stop=True)
                e_t = stream.tile([P, IB], cdt, tag="e", bufs=16,
                                  name=f"e_{h}_{ibx}_{jt}")
                nc.scalar.activation(e_t[:], sc_t[:], Exp,
                                     bias=0.0, scale=SCALE)
                e_ts[jt] = e_t

            def av_step(h, av, pj, e_ts):
                hc = h // HPC
                hh = h % HPC
                e_t = e_ts.pop(pj)
                for x in range(XB):
                    nc.tensor.matmul(
                        av[0:DK + 1, x * SC:(x + 1) * SC],
                        lhsT=v_c[hc][:, pj, hh, :],
                        rhs=e_t[:, x * SC:(x + 1) * SC],
                        start=(pj == 0), stop=(pj == JT - 1))

            def normalize(h, ibx, av):
                hc = h // HPC
                hh = h % HPC
                p0 = hh * DK
                i0 = ibx * IB
                avsb = stream.tile([P, IB], f32, tag="avsb", bufs=2,
                                   name=f"avsb_{h}_{ibx}")
                nc.vector.tensor_copy(avsb[0:DK + 1, :], av[0:DK + 1, :])
                zd = dscratch.tile([1, IB], f32, tag="zd", bufs=2,
                                   name=f"zd_{h}_{ibx}")
                nc.sync.dma_start(zd[:], avsb[DK:DK + 1, :])
                zp = stream.tile([P, ZF], f32, tag="zp", bufs=2,
                                 name=f"zp_{h}_{ibx}")
                nc.sync.dma_start(
                    zp[:], zd[:, :].rearrange("o (p f) -> (o p) f", p=P))
                rzp = stream.tile([P, ZF], f32, tag="rzp", bufs=2,
                                  name=f"rzp_{h}_{ibx}")
                nc.vector.reciprocal(rzp[:], zp[:])
                zi = dscratch.tile([1, IB], f32, tag="zi", bufs=2,
                                   name=f"zi_{h}_{ibx}")
                nc.sync.dma_start(
                    zi[:, :].rearrange("o (p f) -> (o p) f", p=P), rzp[:])
                rzb = stream.tile([P, IB], f32, tag="rzb", bufs=2,
                                  name=f"rzb_{h}_{ibx}")
                nc.sync.dma_start(rzb[0:DK, :],
                                  zi[:, :].to_broadcast((DK, IB)))
                ot = stream.tile([P, IB], cdt, tag="ot", bufs=2,
                                 name=f"ot_{h}_{ibx}")
                nc.vector.tensor_mul(ot[0:DK, :], avsb[0:DK, :],
                                     rzb[0:DK, :])
                nc.sync.dma_start(on_c[hc][p0:p0 + DK, i0:i0 + IB],
                                  ot[0:DK, :])

            # head 0: lookahead block. 16 scores+exp go first (the exp
            # stream starts before vT lands); V-proj chunks and both
            # i-blocks' AV matmuls trail behind.
            e0, e1 = {}, {}
            av0 = psum.tile([P, IB], f32, tag="av", bufs=2, name="av_0_0")
            av1 = psum.tile([P, IB], f32, tag="av", bufs=2, name="av_0_1")
            for jt in range(JT):
                scores_exp(0, 0, jt, e0)
            for pj in range(JT):
                if pj % 4 == 0:
                    v_proj_chunk(range(pj, pj + 4))
                av_step(0, av0, pj, e0)
                scores_exp(0, 1, pj, e1)
                if pj >= 1:
                    av_step(0, av1, pj - 1, e1)
            normalize(0, 0, av0)
            av_step(0, av1, JT - 1, e1)
            normalize(0, 1, av1)

            # heads 1..3: standard trailing flash blocks
            for h in range(1, H_CORE):
                for ibx in range(NIB):
                    av = psum.tile([P, IB], f32, tag="av", bufs=2,
                                   name=f"av_{h}_{ibx}")
                    e_ts = {}
                    for jt in range(JT + 1):
                        if jt < JT:
                            scores_exp(h, ibx, jt, e_ts)
                        if jt >= 1:
                            av_step(h, av, jt - 1, e_ts)
                    normalize(h, ibx, av)

            # ---- output projection ----
            for n in range(NOUT):
                for iq in range(SJ):
                    idx = n * SJ + iq
                    ps = psum.tile([P, IB], f32, tag=("sc", "av")[idx % 2],
                                   bufs=2, name=f"ps_o_{n}_{iq}")
                    for c in range(HC):
                        for x in range(XB):
                            nc.tensor.matmul(
                                ps[:, x * SC:(x + 1) * SC],
                                lhsT=wo_sb[:, c, n * P:(n + 1) * P],
                                rhs=on_c[c][:, iq * IB + x * SC:
                                            iq * IB + (x + 1) * SC],
                                start=(c == 0), stop=(c == HC - 1))
                    o_sb = stream.tile([P, IB], odt, tag="osb", bufs=4,
                                       name=f"o_sb_{n}_{iq}")
                    if idx % 2 == 0:
                        nc.scalar.activation(o_sb[:], ps[:], Ident,
                                             bias=bo_sb[:, n:n + 1],
                                             scale=1.0)
                    else:
                        nc.vector.tensor_add(
                            o_sb[:], ps[:],
                            bo_sb[:, n:n + 1].to_broadcast((P, IB)))
                    nc.sync.dma_start(
                        outT[n * P:(n + 1) * P, iq * IB:(iq + 1) * IB],
                        o_sb[:])

    nc.finalize()
    return nc


def make_in_maps(query, key, value, Wq, bq, Wk, bk, Wv, bv, Wo, bo,
                 dtype="bf16"):
    """Shard full inputs into the 8 per-core input dicts."""
    f = lambda a: np.ascontiguousarray(np.asarray(a, dtype=np.float32))
    KT = D // P
    HC = DH // P
    NOUT = D // P
    query, key, value = f(query), f(key), f(value)
    Wq, Wk, Wv, Wo = f(Wq), f(Wk), f(Wv), f(Wo)
    bq, bk, bv, bo = f(bq), f(bk), f(bv), f(bo)
    if dtype == "bf16":
        import ml_dtypes
        cvt = lambda a: np.ascontiguousarray(a.astype(ml_dtypes.bfloat16))
    else:
        cvt = np.ascontiguousarray

    def wmajor(w):   # [D, DH] -> [P, KT, DH] partition-major
        return cvt(w.reshape(KT, P, DH).transpose(1, 0, 2))

    in_maps = []
    for core in range(N_CORES):
        b, g = core // GROUPS, core % GROUPS
        sl = slice(g * DH, (g + 1) * DH)
        in_maps.append({
            "qT": cvt(query[b].T),
            "kTd": cvt(key[b].T),
            "vT": cvt(value[b].T),
            "wq": wmajor(Wq[:, sl]),
            "wk": wmajor(Wk[:, sl]),
            "wv": wmajor(Wv[:, sl]),
            "wo": cvt(Wo[sl, :].reshape(HC, P, D).transpose(1, 0, 2)),
            "bq": np.ascontiguousarray(bq[sl].reshape(HC, P).T),
            "bk": np.ascontiguousarray(bk[sl].reshape(HC, P).T),
            "bvb": np.ascontiguousarray(
                np.broadcast_to(bv[sl].reshape(H_CORE, DK)[None],
                                (P, H_CORE, DK))),
            "bo": (np.ascontiguousarray(bo.reshape(NOUT, P).T)
                   if g == 0 else np.zeros((P, NOUT), np.float32)),
        })
    return in_maps


# test hooks (ignored by the harness)
TRACE = False
LAST_RESULT = None
DTYPE = "bf16"
_NC_CACHE = {}


def kernel(query, key, value, Wq, bq, Wk, bk, Wv, bv, Wo, bo):
    global LAST_RESULT
    from concourse.bass_utils import run_bass_kernel_spmd

    if DTYPE not in _NC_CACHE:
        _NC_CACHE[DTYPE] = build_nc(dtype=DTYPE)
    nc = _NC_CACHE[DTYPE]

    in_maps = make_in_maps(query, key, value, Wq, bq, Wk, bk, Wv, bv, Wo, bo,
                           dtype=DTYPE)
    kwargs = {}
    if TRACE:
        kwargs = dict(trace=True, trace_cores=[0])
    res = run_bass_kernel_spmd(nc, in_maps, core_ids=list(range(N_CORES)),
                               **kwargs)
    LAST_RESULT = res

    out = np.zeros((B, S, D), np.float32)
    for core in range(N_CORES):
        b = core // GROUPS
        out[b] += np.asarray(res.results[core]["outT"],
                             dtype=np.float32).T
    return out


# revision 12
# speedup vs baseline: 1.3309x; 1.0392x over previous
"""Multi-head attention, tensor-parallel across 8 Trainium2 NeuronCores.

Sharding: core = (batch b, head-group g) with b in {0,1}, g in {0..3}.
Each core computes 4 heads (a 256-wide slice of the head dimension) for one
batch element:
  Q^T/K^T = Wq/Wk slice^T-projections of query/key (kept transposed: [dh, s])
  V       = value @ Wv slice (natural [s, dh]), with an appended ones column
  S^T     = K^T-chunk.T @ Q^T-chunk per head  -> scores transposed [j, i]
  E       = exp(S^T * scale)                  (no max subtraction; scores ~N(0,1))
  [O^T;Z] = V'.T @ E  accumulated over j      (ones column yields Z = sum_j E)
  Onorm^T = O^T * (1/Z) broadcast
  outT    = Wo-slice.T @ Onorm^T (+ bo on group-0 cores only)
Host: transposes activations into [D, S] per core, pre-arranges weights into
partition-major layout (so every DMA line is >=2KB), and sums the 4 group
partials per batch (the "all-reduce" of the output projection).

Schedule (per core):
  - All input DMAs issue up front in priority order (wk/wq, kT, qT, wv, vT, wo)
    so the Sync engine never head-of-line-blocks a load behind a dependent
    store; transfers use full/half row granules (2-4KB per-partition lines).
  - K/Q projections accumulate kt-major so the PE starts as rows land.
  - Attention is ACT(exp)-bound: head 0 runs a "lookahead" block that emits
    16 scores+exp first (deep e-tile buffering) so the exp stream starts
    before V has even arrived; V projection chunks and both i-blocks' AV
    matmuls are interleaved behind it.
  - Softmax denominators are normalized via a packed [128,8] reciprocal
    (1/Z gathered through DRAM) instead of 1-partition reciprocal ops.
"""

import numpy as np

# Problem shape (hardcoded per the harness contract).
B, S, D, H = 2, 2048, 1024, 16
DK = D // H              # 64 head dim
N_CORES = 8
GROUPS = N_CORES // B    # 4 head-groups
DH = D // GROUPS         # 256 head-dims per core (4 heads)
H_CORE = DH // DK        # 4 heads per core
SCALE = 1.0 / float(np.sqrt(DK))

P = 128                  # SBUF/PSUM partitions
SC = 512                 # matmul moving-dim chunk (one PSUM bank of fp32)
IB = 1024                # flash i-block / exp granule / DMA granule


def build_nc(dtype="bf16"):
    """Build the per-core Bass module (same NEFF for all 8 cores)."""
    import concourse.bacc as bacc
    import concourse.mybir as mybir
    import concourse.tile as tile

    f32 = mybir.dt.float32
    f32r = mybir.dt.float32r
    bf16 = mybir.dt.bfloat16
    Exp = mybir.ActivationFunctionType.Exp
    Ident = mybir.ActivationFunctionType.Identity

    KT = D // P                    # 8 contraction tiles for projections
    HC = DH // P                   # 2 head-dim chunks
    HPC = P // DK                  # 2 heads per chunk
    JT = S // P                    # 16 j tiles
    NIB = S // IB                  # 2 i blocks
    SJ = S // IB                   # 2 column halves (DMA/proj granule)
    NOUT = D // P                  # 8 output row chunks
    XB = IB // SC                  # 2 moving chunks per i block
    ZF = IB // P                   # 8 z values per partition when packed

    cdt = {"f32r": f32r, "bf16": bf16, "f32": f32}[dtype]
    odt = f32 if dtype == "f32" else bf16

    nc = bacc.Bacc("TRN2", target_bir_lowering=False, debug=False)

    qT = nc.dram_tensor("qT", [D, S], cdt, kind="ExternalInput")
    kTd = nc.dram_tensor("kTd", [D, S], cdt, kind="ExternalInput")
    vT = nc.dram_tensor("vT", [D, S], cdt, kind="ExternalInput")
    wq = nc.dram_tensor("wq", [P, KT, DH], cdt, kind="ExternalInput")
    wk = nc.dram_tensor("wk", [P, KT, DH], cdt, kind="ExternalInput")
    wv = nc.dram_tensor("wv", [P, KT, DH], cdt, kind="ExternalInput")
    wo = nc.dram_tensor("wo", [P, HC, D], cdt, kind="ExternalInput")
    bq = nc.dram_tensor("bq", [P, HC], f32, kind="ExternalInput")
    bk = nc.dram_tensor("bk", [P, HC], f32, kind="ExternalInput")
    bvb = nc.dram_tensor("bvb", [P, H_CORE, DK], f32, kind="ExternalInput")
    bo = nc.dram_tensor("bo", [P, NOUT], f32, kind="ExternalInput")
    outT = nc.dram_tensor("outT", [D, S], odt, kind="ExternalOutput")

    with tile.TileContext(nc) as tc:
        with (
            tc.tile_pool(name="const", bufs=1) as cpool,
            tc.tile_pool(name="pers", bufs=1) as pers,
            tc.tile_pool(name="stream", bufs=1) as stream,
            tc.tile_pool(name="psum", bufs=1, space="PSUM") as psum,
            tc.tile_pool(name="dscratch", bufs=1, space="DRAM") as dscratch,
        ):
            # ---- ACT exp-table warm-up (overlaps the input DMA stream) ----
            warm = stream.tile([1, 8], f32, name="warm")
            warm_o = stream.tile([1, 8], f32, name="warm_o")
            nc.vector.memset(warm[:], 0.0)
            nc.scalar.activation(warm_o[:], warm[:], Exp, bias=0.0, scale=1.0)

            # ---- constants + inputs, DMA'd in priority order ----
            wk_sb = cpool.tile([P, KT, DH], cdt, name="wk_sb")
            wq_sb = cpool.tile([P, KT, DH], cdt, name="wq_sb")
            wv_sb = cpool.tile([P, KT, DH], cdt, name="wv_sb")
            wo_sb = cpool.tile([P, HC, D], cdt, name="wo_sb")
            bq_sb = cpool.tile([P, HC], f32, name="bq_sb")
            bk_sb = cpool.tile([P, HC], f32, name="bk_sb")
            bvb_sb = cpool.tile([P, H_CORE, DK], f32, name="bvb_sb")
            bo_sb = cpool.tile([P, NOUT], f32, name="bo_sb")

            nc.sync.dma_start(wk_sb[:], wk[:, :, :])
            nc.sync.dma_start(wq_sb[:], wq[:, :, :])
            nc.sync.dma_start(bk_sb[:], bk[:, :])
            nc.sync.dma_start(bq_sb[:], bq[:, :])

            # kT halves (tag shared with vT rows: vT reuses the slots after
            # the K projection has consumed them)
            krow = [[None] * SJ for _ in range(KT)]
            for kt in range(KT):
                for hf in range(SJ):
                    t = stream.tile([P, IB], cdt, tag="xrow", bufs=16,
                                    name=f"krow{kt}_{hf}")
                    nc.sync.dma_start(
                        t[:], kTd[kt * P:(kt + 1) * P, hf * IB:(hf + 1) * IB])
                    krow[kt][hf] = t

            # qT halves: all of half 0 first so Q-proj wave A starts early
            qrow = [[None] * KT for _ in range(SJ)]
            for hf in range(SJ):
                for kt in range(KT):
                    t = stream.tile([P, IB], cdt, tag="qrow", bufs=16,
                                    name=f"qrow{kt}_{hf}")
                    nc.sync.dma_start(
                        t[:], qT[kt * P:(kt + 1) * P, hf * IB:(hf + 1) * IB])
                    qrow[hf][kt] = t

            # vT triggers go on the Scalar HWDGE queue: they carry WAR deps
            # on the K-projection (slot reuse), and on the Sync queue they
            # would head-of-line-block every later internal DMA. The Scalar
            # engine is idle until the exp stream starts, so it can absorb
            # the wait for free.
            nc.scalar.dma_start(wv_sb[:], wv[:, :, :])
            nc.scalar.dma_start(bvb_sb[:], bvb[:, :, :])
            vrow = [[None] * SJ for _ in range(KT)]
            for hf in range(SJ):
                for kt in range(KT):
                    t = stream.tile([P, IB], cdt, tag="xrow", bufs=16,
                                    name=f"vrow{kt}_{hf}")
                    nc.scalar.dma_start(
                        t[:], vT[kt * P:(kt + 1) * P, hf * IB:(hf + 1) * IB])
                    vrow[kt][hf] = t

            nc.sync.dma_start(wo_sb[:], wo[:, :, :])
            nc.sync.dma_start(bo_sb[:], bo[:, :])

            # ---- persistent activations ----
            # Q^T/K^T live per head on partitions 64-127 (base-64 K=64
            # matmuls sustain full rate; base-0 ones run at half rate).
            qt_h = [pers.tile([P, S], cdt, name=f"qth{h}")
                    for h in range(H_CORE)]
            kt_h = [pers.tile([P, S], cdt, name=f"kth{h}")
                    for h in range(H_CORE)]
            v_c = [pers.tile([P, JT, HPC, DK + 1], cdt, name=f"v{c}")
                   for c in range(HC)]
            on_c = [pers.tile([P, S], cdt, name=f"on{c}") for c in range(HC)]

            for c in range(HC):
                ones_ap = v_c[c][:, :, :, DK:DK + 1]
                if dtype == "f32r":
                    ones_ap = ones_ap.bitcast(f32)
                nc.vector.memset(ones_ap, 1.0)

            # Zero the unused lower 64 partitions of Q^T/K^T so the scores
            # matmuls can run K=128 full-array mode (no 64-row tile-mode
            # switches against the K=128 AV matmuls, and the 128x128 bf16
            # stationary operand is fast-weight-load eligible).
            for h in range(H_CORE):
                nc.vector.memset(qt_h[h][0:DK, :], 0.0)
                nc.vector.memset(kt_h[h][0:DK, :], 0.0)

            # ---- K/Q projections (kt-major accumulation per column half) ----
            def qk_proj(rows, w_sb, b_sb, dst, pname):
                for sj in range(SJ):
                    pss = []
                    for c in range(HC):
                        ps = psum.tile([P, IB], f32, tag=("sc", "av")[c],
                                       bufs=2, name=f"ps_{pname}_{sj}_{c}")
                        pss.append(ps)
                    for kt in range(KT):
                        for c in range(HC):
                            for x in range(XB):
                                nc.tensor.matmul(
                                    pss[c][:, x * SC:(x + 1) * SC],
                                    lhsT=w_sb[:, kt, c * P:(c + 1) * P],
                                    rhs=rows(kt, sj)[:, x * SC:(x + 1) * SC],
                                    start=(kt == 0), stop=(kt == KT - 1))
                    for c in range(HC):
                        stg = stream.tile([P, IB], cdt, tag="stg", bufs=3,
                                          name=f"stg_{pname}_{sj}_{c}")
                        nc.vector.tensor_add(
                            stg[:], pss[c][:],
                            b_sb[:, c:c + 1].to_broadcast((P, IB)))
                        ssl = slice(sj * IB, (sj + 1) * IB)
                        nc.sync.dma_start(dst[c * HPC][DK:P, ssl],
                                          stg[0:DK, :])
                        nc.sync.dma_start(dst[c * HPC + 1][DK:P, ssl],
                                          stg[DK:P, :])

            qk_proj(lambda kt, sj: krow[kt][sj], wk_sb, bk_sb, kt_h, "k")
            qk_proj(lambda kt, sj: qrow[sj][kt], wq_sb, bq_sb, qt_h, "q")

            # ---- V projection (natural [s, dh]), emitted in chunks that are
            # interleaved into head 0's attention stream ----
            def v_proj_chunk(jts):
                for jt in jts:
                    hf, sub = divmod(jt, IB // P)
                    ps = psum.tile([P, DH], f32, tag="sc", bufs=2,
                                   name=f"ps_v_{jt}")
                    for kt in range(KT):
                        nc.tensor.matmul(
                            ps[:],
                            lhsT=vrow[kt][hf][:, sub * P:(sub + 1) * P],
                            rhs=wv_sb[:, kt, :],
                            start=(kt == 0), stop=(kt == KT - 1))
                    for c in range(HC):
                        nc.vector.tensor_add(
                            v_c[c][:, jt, :, 0:DK],
                            ps[:, c * P:(c + 1) * P].rearrange(
                                "p (h d) -> p h d", d=DK),
                            bvb_sb[:, c * HPC:(c + 1) * HPC, :])

            # ---- attention (flash over j, scores transposed) ----
            def scores_exp(h, ibx, jt, e_ts):
                i0 = ibx * IB
                sc_t = psum.tile([P, IB], f32, tag="sc", bufs=2,
                                 name=f"sc_{h}_{ibx}_{jt}")
                for x in range(XB):
                    nc.tensor.matmul(
                        sc_t[:, x * SC:(x + 1) * SC],
                        lhsT=kt_h[h][0:P, jt * P:(jt + 1) * P],
                        rhs=qt_h[h][0:P, i0 + x * SC:i0 + (x + 1) * SC],
                        start=True, stop=True)
                e_t = stream.tile([P, IB], cdt, tag="e", bufs=16,
                                  name=f"e_{h}_{ibx}_{jt}")
                nc.scalar.activation(e_t[:], sc_t[:], Exp,
                                     bias=0.0, scale=SCALE)
                e_ts[jt] = e_t

            def av_step(h, av, pj, e_ts):
                hc = h // HPC
                hh = h % HPC
                e_t = e_ts.pop(pj)
                for x in range(XB):
                    nc.tensor.matmul(
                        av[0:DK + 1, x * SC:(x + 1) * SC],
                        lhsT=v_c[hc][:, pj, hh, :],
                        rhs=e_t[:, x * SC:(x + 1) * SC],
                        start=(pj == 0), stop=(pj == JT - 1))

            def normalize(h, ibx, av):
                hc = h // HPC
                hh = h % HPC
                p0 = hh * DK
                i0 = ibx * IB
                avsb = stream.tile([P, IB], f32, tag="avsb", bufs=2,
                                   name=f"avsb_{h}_{ibx}")
                nc.vector.tensor_copy(avsb[0:DK + 1, :], av[0:DK + 1, :])
                zd = dscratch.tile([1, IB], f32, tag="zd", bufs=2,
                                   name=f"zd_{h}_{ibx}")
                nc.sync.dma_start(zd[:], avsb[DK:DK + 1, :])
                zp = stream.tile([P, ZF], f32, tag="zp", bufs=2,
                                 name=f"zp_{h}_{ibx}")
                nc.sync.dma_start(
                    zp[:], zd[:, :].rearrange("o (p f) -> (o p) f", p=P))
                rzp = stream.tile([P, ZF], f32, tag="rzp", bufs=2,
                                  name=f"rzp_{h}_{ibx}")
                nc.vector.reciprocal(rzp[:], zp[:])
                zi = dscratch.tile([1, IB], f32, tag="zi", bufs=2,
                                   name=f"zi_{h}_{ibx}")
                nc.sync.dma_start(
                    zi[:, :].rearrange("o (p f) -> (o p) f", p=P), rzp[:])
                rzb = stream.tile([P, IB], f32, tag="rzb", bufs=2,
                                  name=f"rzb_{h}_{ibx}")
                nc.sync.dma_start(rzb[0:DK, :],
                                  zi[:, :].to_broadcast((DK, IB)))
                ot = stream.tile([P, IB], cdt, tag="ot", bufs=2,
                                 name=f"ot_{h}_{ibx}")
                nc.vector.tensor_mul(ot[0:DK, :], avsb[0:DK, :],
                                     rzb[0:DK, :])
                nc.sync.dma_start(on_c[hc][p0:p0 + DK, i0:i0 + IB],
                                  ot[0:DK, :])

            def oproj_chunk(n, iq):
                idx = n * SJ + iq
                ps = psum.tile([P, IB], f32, tag="sc", bufs=2,
                               name=f"ps_o_{n}_{iq}")
                for c in range(HC):
                    for x in range(XB):
                        nc.tensor.matmul(
                            ps[:, x * SC:(x + 1) * SC],
                            lhsT=wo_sb[:, c, n * P:(n + 1) * P],
                            rhs=on_c[c][:, iq * IB + x * SC:
                                        iq * IB + (x + 1) * SC],
                            start=(c == 0), stop=(c == HC - 1))
                o_sb = stream.tile([P, IB], odt, tag="osb", bufs=4,
                                   name=f"o_sb_{n}_{iq}")
                if iq == 1 and idx % 2 == 0:
                    # tail chunks: ACT is idle once the exp stream is done
                    nc.scalar.activation(o_sb[:], ps[:], Ident,
                                         bias=bo_sb[:, n:n + 1],
                                         scale=1.0)
                else:
                    nc.vector.tensor_add(
                        o_sb[:], ps[:],
                        bo_sb[:, n:n + 1].to_broadcast((P, IB)))
                nc.sync.dma_start(
                    outT[n * P:(n + 1) * P, iq * IB:(iq + 1) * IB],
                    o_sb[:])

            # head 0: lookahead block. 16 scores+exp go first (the exp
            # stream starts before vT lands); V-projection single-jt chunks
            # ride the PE slack of the exp-paced lookahead steps, and both
            # i-blocks' AV matmuls trail behind.
            e0, e1 = {}, {}
            av0 = psum.tile([P, IB], f32, tag="av", bufs=2, name="av_0_0")
            av1 = psum.tile([P, IB], f32, tag="av", bufs=2, name="av_0_1")
            for jt in range(JT):
                scores_exp(0, 0, jt, e0)
                if jt >= 6:
                    v_proj_chunk([jt - 6])          # V jts 0..9
            for pj in range(JT):
                if pj < 6:
                    v_proj_chunk([pj + 10])         # V jts 10..15
                av_step(0, av0, pj, e0)
                scores_exp(0, 1, pj, e1)
                if pj >= 1:
                    av_step(0, av1, pj - 1, e1)
            normalize(0, 0, av0)
            av_step(0, av1, JT - 1, e1)
            normalize(0, 1, av1)

            # heads 1..3: standard trailing flash blocks. The final block
            # carries the first half of the output projection in its slack.
            for h in range(1, H_CORE):
                for ibx in range(NIB):
                    last = (h == H_CORE - 1 and ibx == NIB - 1)
                    av = psum.tile([P, IB], f32, tag="av", bufs=2,
                                   name=f"av_{h}_{ibx}")
                    e_ts = {}
                    for jt in range(JT + 1):
                        if jt < JT:
                            scores_exp(h, ibx, jt, e_ts)
                        if jt >= 1:
                            av_step(h, av, jt - 1, e_ts)
                        if last and jt >= 9:
                            oproj_chunk(jt - 9, 0)  # n 0..7, i-half 0
                    normalize(h, ibx, av)

            # ---- output projection (second i-half) ----
            for n in range(NOUT):
                oproj_chunk(n, 1)

    nc.finalize()
    return nc


def make_in_maps(query, key, value, Wq, bq, Wk, bk, Wv, bv, Wo, bo,
                 dtype="bf16"):
    """Shard full inputs into the 8 per-core input dicts."""
    f = lambda a: np.ascontiguousarray(np.asarray(a, dtype=np.float32))
    KT = D // P
    HC = DH // P
    NOUT = D // P
    query, key, value = f(query), f(key), f(value)
    Wq, Wk, Wv, Wo = f(Wq), f(Wk), f(Wv), f(Wo)
    bq, bk, bv, bo = f(bq), f(bk), f(bv), f(bo)
    if dtype == "bf16":
        import ml_dtypes
        cvt = lambda a: np.ascontiguousarray(a.astype(ml_dtypes.bfloat16))
    else:
        cvt = np.ascontiguousarray

    def wmajor(w):   # [D, DH] -> [P, KT, DH] partition-major
        return cvt(w.reshape(KT, P, DH).transpose(1, 0, 2))

    in_maps = []
    for core in range(N_CORES):
        b, g = core // GROUPS, core % GROUPS
        sl = slice(g * DH, (g + 1) * DH)
        in_maps.append({
            "qT": cvt(query[b].T),
            "kTd": cvt(key[b].T),
            "vT": cvt(value[b].T),
            "wq": wmajor(Wq[:, sl]),
            "wk": wmajor(Wk[:, sl]),
            "wv": wmajor(Wv[:, sl]),
            "wo": cvt(Wo[sl, :].reshape(HC, P, D).transpose(1, 0, 2)),
            "bq": np.ascontiguousarray(bq[sl].reshape(HC, P).T),
            "bk": np.ascontiguousarray(bk[sl].reshape(HC, P).T),
            "bvb": np.ascontiguousarray(
                np.broadcast_to(bv[sl].reshape(H_CORE, DK)[None],
                                (P, H_CORE, DK))),
            "bo": (np.ascontiguousarray(bo.reshape(NOUT, P).T)
                   if g == 0 else np.zeros((P, NOUT), np.float32)),
        })
    return in_maps


# test hooks (ignored by the harness)
TRACE = False
LAST_RESULT = None
DTYPE = "bf16"
_NC_CACHE = {}


def kernel(query, key, value, Wq, bq, Wk, bk, Wv, bv, Wo, bo):
    global LAST_RESULT
    from concourse.bass_utils import run_bass_kernel_spmd

    if DTYPE not in _NC_CACHE:
        _NC_CACHE[DTYPE] = build_nc(dtype=DTYPE)
    nc = _NC_CACHE[DTYPE]

    in_maps = make_in_maps(query, key, value, Wq, bq, Wk, bk, Wv, bv, Wo, bo,
                           dtype=DTYPE)
    kwargs = {}
    if TRACE:
        kwargs = dict(trace=True, trace_cores=[0])
    res = run_bass_kernel_spmd(nc, in_maps, core_ids=list(range(N_CORES)),
                               **kwargs)
    LAST_RESULT = res

    out = np.zeros((B, S, D), np.float32)
    for core in range(N_CORES):
        b = core // GROUPS
        out[b] += np.asarray(res.results[core]["outT"],
                             dtype=np.float32).T
    return out
